# revision 5
# baseline (speedup 1.0000x reference)
"""Trainium2 Bass kernel for the 4-layer GCN diffusion denoiser (gnn_message_passing).

Strategy (8 NeuronCores, SPMD single program):
  - Nodes sharded 12500/core (padded to 12544 = 98*128). Edges routed to the core
    owning their dst node, bucketed into 512-node windows.
  - Per layer, per-node features Hs = dinv * (X @ W) are stored as a bf16
    [100352, 128] table (row-padded feature dim), AllGather'ed across cores.
  - Aggregation per 512-node window: bulk indirect row gathers (dma_gather,
    int16 indices into 4 x 25088-row bucket views), segment-sum via PE matmuls
    against on-device-built one-hot matrices (iota == dst_local) * dinv_dst,
    self-loop via HsSelf x diag(dinv) matmul, Silu(agg + bias) on ScalarE.
  - The next layer's H-matmul consumes the transposed activation tile directly
    (lhsT = x'^T), so no transposes are needed except for the final output.

All cross-core communication is 4 AllGathers (one per layer boundary).
"""

import math
import sys
import types

import numpy as np

_N, _E, _D, _G = 100000, 1000000, 64, 128
_NCORES = 8
_SL = _N // _NCORES          # 12500 real nodes per core
_SLP = 12544                 # padded per-core slice (98*128)
_NP = _SLP * _NCORES         # 100352 padded table rows
_NBUCK = 4
_BUCK = _NP // _NBUCK        # 25088 rows per gather bucket (int16-addressable)
_WIN = 512
_NWIN = (_SLP + _WIN - 1) // _WIN    # 25 windows (last is 256 nodes)
_NSUB = _SLP // 128          # 98 sub-tiles of 128 nodes
_F = 128
_PAD_DST = 99999.0
_MAX_CHUNKS_PER_CALL = 4     # <=512 rows per dma_gather (descriptor ring limit)

_compiled = {}


def _install_profile_shim():
    """Register the NTFF profile hook missing from this image's antenv."""
    try:
        import antenv
        from trn_agent_boot.trn_boot import _ntff_profile_via_ctypes
    except ImportError:
        return
    if "antenv.axon_hooks" in sys.modules:
        return
    mod = types.ModuleType("antenv.axon_hooks")
    hook = _ntff_profile_via_ctypes("/opt/axon/libaxon_pjrt.so")
    mod.get_axon_ntff_profile_hook = lambda: hook
    mod.set_axon_ntff_profile_hook = lambda h: None
    sys.modules["antenv.axon_hooks"] = mod
    antenv.axon_hooks = mod


def _prep(inputs):
    """Host-side metadata build: edge bucketing, gather indices, selectors."""
    src = np.asarray(inputs["edge_index"][0], dtype=np.int64)
    dst = np.asarray(inputs["edge_index"][1], dtype=np.int64)
    deg = np.bincount(dst, minlength=_N).astype(np.float32) + 1.0
    dinv = (1.0 / np.sqrt(deg)).astype(np.float32)

    node_ids = np.arange(_N)
    node_row = (_SLP * (node_ids // _SL) + (node_ids % _SL)).astype(np.int64)

    core_of = dst // _SL
    dloc = dst % _SL
    w_of = dloc // _WIN
    dstloc = (dloc % _WIN).astype(np.float32)
    srow = node_row[src]
    buck = srow // _BUCK
    idx16 = (srow % _BUCK).astype(np.int16)
    dinv_dst = dinv[dst]

    order = np.lexsort((buck, w_of, core_of))
    buck_s, core_s, w_s = buck[order], core_of[order], w_of[order]
    idx16_s, dstloc_s, dinvd_s = idx16[order], dstloc[order], dinv_dst[order]

    key = ((core_s * _NWIN) + w_s) * _NBUCK + buck_s
    nkeys = _NCORES * _NWIN * _NBUCK
    counts = np.bincount(key, minlength=nkeys)
    runlen = counts.reshape(_NCORES, _NWIN, _NBUCK)
    cpb = np.maximum(1, np.ceil(runlen.max(axis=0) / 128).astype(np.int64))
    slots_per_call = cpb * 128
    nchunks_w = cpb.sum(axis=1)
    tot_chunks = int(nchunks_w.sum())
    tot_slots = tot_chunks * 128

    call_off = np.zeros((_NWIN, _NBUCK), np.int64)
    acc = 0
    for w in range(_NWIN):
        for b in range(_NBUCK):
            call_off[w, b] = acc
            acc += slots_per_call[w, b]

    run_start = np.zeros(nkeys + 1, np.int64)
    np.cumsum(counts, out=run_start[1:])

    is_norm = np.zeros(_N, bool)
    is_norm[np.asarray(inputs["train_norm"])] = True
    is_anm = np.zeros(_N, bool)
    is_anm[np.asarray(inputs["train_anm"])] = True

    noise = np.asarray(inputs["noise_x"], np.float32)
    t_val = float(np.asarray(inputs["t"]).reshape(-1)[0])
    half = _D // 2
    freqs = np.exp(
        np.arange(half, dtype=np.float32) * (-math.log(10000.0) / (half - 1))
    ).astype(np.float32)

    w_ = {m: np.asarray(inputs[m], np.float32) for m in
          ["w0", "b0", "w1", "b1", "w2", "b2", "w3", "b3",
           "time_w1", "time_b1", "time_w2", "time_b2", "label_emb"]}
    w1p = np.zeros((128, 128), np.float32); w1p[:, :64] = w_["w1"]
    w2p = np.zeros((128, 128), np.float32); w2p[:64, :] = w_["w2"]
    w3ap = np.zeros((128, 128), np.float32); w3ap[:, :64] = w_["w3"][:128]
    w3bp = np.zeros((128, 128), np.float32); w3bp[:, :64] = w_["w3"][128:]
    b1p = np.zeros((128, 1), np.float32); b1p[:64, 0] = w_["b1"]
    b3p = np.zeros((128, 1), np.float32); b3p[:64, 0] = w_["b3"]

    shared = {
        "w0": w_["w0"],                       # [64, 128]
        "w1p": w1p, "w2p": w2p, "w3ap": w3ap, "w3bp": w3bp,
        "b0c": w_["b0"].reshape(128, 1).astype(np.float32),
        "b1c": b1p,
        "b2c": w_["b2"].reshape(128, 1).astype(np.float32),
        "b3c": b3p,
        "tw1": w_["time_w1"], "tw2": w_["time_w2"],
        "tb1": w_["time_b1"].reshape(64, 1).astype(np.float32),
        "tb2": w_["time_b2"].reshape(64, 1).astype(np.float32),
        "e0row": w_["label_emb"][0].reshape(1, 64).astype(np.float32),
        "e1row": w_["label_emb"][1].reshape(1, 64).astype(np.float32),
        "rsin": (np.mod(t_val * freqs + np.pi, 2 * np.pi) - np.pi).reshape(32, 1).astype(np.float32),
        "rcos": (np.mod(t_val * freqs + np.pi / 2 + np.pi, 2 * np.pi) - np.pi).reshape(32, 1).astype(np.float32),
        "ones1": np.ones((1, 128), np.float32),
    }

    in_maps = []
    for k in range(_NCORES):
        idx_slots = np.zeros(tot_slots, np.int16)
        dstloc_slots = np.full(tot_slots, _PAD_DST, np.float32)
        dinvd_slots = np.zeros(tot_slots, np.float32)
        for w in range(_NWIN):
            for b in range(_NBUCK):
                kk = ((k * _NWIN) + w) * _NBUCK + b
                s0, s1 = run_start[kk], run_start[kk + 1]
                o = call_off[w, b]
                idx_slots[o:o + (s1 - s0)] = idx16_s[s0:s1]
                dstloc_slots[o:o + (s1 - s0)] = dstloc_s[s0:s1]
                dinvd_slots[o:o + (s1 - s0)] = dinvd_s[s0:s1]
        wrapped = np.tile(idx_slots.reshape(-1, 16).T, (8, 1))
        dl = dstloc_slots.reshape(-1, 128).T.copy()
        dvd = dinvd_slots.reshape(-1, 128).T.copy()

        nodes = np.arange(_SLP) + k * _SL
        nodes_c = np.minimum(nodes, _N - 1)
        sd = dinv[nodes_c].copy()
        sd[np.arange(_SLP) >= _SL] = 1.0
        selfdinv = sd.reshape(_NSUB, 128).T.copy()

        s_n = np.zeros(_SLP, np.float32)
        s_a = np.zeros(_SLP, np.float32)
        real = np.arange(_SLP) < _SL
        s_n[real] = is_norm[nodes[real]]
        s_a[real] = is_anm[nodes[real]] & ~is_norm[nodes[real]]

        nz = np.zeros((_SLP, _D), np.float32)
        nz[:_SL] = noise[k * _SL:(k + 1) * _SL]

        m = dict(shared)
        m.update({
            "midx": wrapped,
            "mdstl": dl,
            "mdstr": (dl - 256.0).astype(np.float32),
            "mdinvd": dvd,
            "mself": selfdinv,
            "ms0": s_n.reshape(_NSUB, 128).T.copy(),
            "ms1": s_a.reshape(_NSUB, 128).T.copy(),
            "noise": nz,
        })
        in_maps.append(m)

    return in_maps, cpb, call_off, nchunks_w, tot_chunks, tot_slots


def _build(cpb, call_off, nchunks_w, tot_chunks, tot_slots):
    import concourse.bass as bass
    import concourse.bacc as bacc
    import concourse.tile as tile
    from concourse import mybir
    from concourse.masks import make_identity

    f32 = mybir.dt.float32
    bf16 = mybir.dt.bfloat16
    AT = mybir.ActivationFunctionType
    OP = mybir.AluOpType

    nc = bacc.Bacc("TRN2", target_bir_lowering=False, debug=False,
                   num_devices=_NCORES, dynamic_dma_scratch_size=32768,
                   num_swdge_queues=4)

    din = {}
    def dt_in(name, shape, dt):
        din[name] = nc.dram_tensor(name, list(shape), dt, kind="ExternalInput")
        return din[name]

    dt_in("noise", (_SLP, _D), f32)
    dt_in("midx", (128, tot_slots // 16), mybir.dt.int16)
    dt_in("mdstl", (128, tot_chunks), f32)
    dt_in("mdstr", (128, tot_chunks), f32)
    dt_in("mdinvd", (128, tot_chunks), f32)
    dt_in("mself", (128, _NSUB), f32)
    dt_in("ms0", (128, _NSUB), f32)
    dt_in("ms1", (128, _NSUB), f32)
    dt_in("w0", (64, 128), f32)
    for nm in ["w1p", "w2p", "w3ap", "w3bp"]:
        dt_in(nm, (128, 128), f32)
    for nm in ["b0c", "b1c", "b2c", "b3c"]:
        dt_in(nm, (128, 1), f32)
    dt_in("tw1", (64, 64), f32)
    dt_in("tw2", (64, 64), f32)
    dt_in("tb1", (64, 1), f32)
    dt_in("tb2", (64, 1), f32)
    dt_in("e0row", (1, 64), f32)
    dt_in("e1row", (1, 64), f32)
    dt_in("rsin", (32, 1), f32)
    dt_in("rcos", (32, 1), f32)
    dt_in("ones1", (1, 128), f32)
    out_d = nc.dram_tensor("out", [_SLP, 64], f32, kind="ExternalOutput")

    wnames = ["w1p", "w2p", None]  # H-matmul weights for layers 0,1 (2 handled via w3a/w3b)
    bnames = ["b0c", "b1c", "b2c", "b3c"]

    qctr = [0]
    def next_q():
        q = qctr[0] % 4
        qctr[0] += 1
        return q

    with tile.TileContext(nc) as tc:
        with tc.tile_pool(name="consts", bufs=1) as cp, \
             tc.tile_pool(name="meta", bufs=1) as mp, \
             tc.tile_pool(name="dram", bufs=1, space="DRAM") as dram, \
             tc.tile_pool(name="g", bufs=3) as gp, \
             tc.tile_pool(name="oh", bufs=6) as ohp, \
             tc.tile_pool(name="small", bufs=4) as sp, \
             tc.tile_pool(name="xt", bufs=2) as xtp, \
             tc.tile_pool(name="psA", bufs=2, space="PSUM") as psA, \
             tc.tile_pool(name="psB", bufs=3, space="PSUM") as psB, \
             tc.tile_pool(name="psC", bufs=2, space="PSUM") as psC:

            # ---- constants / metadata into SBUF ----
            def load(name, shape, dt, pool=cp):
                t = pool.tile(list(shape), dt, tag=name, name=name)
                nc.sync.dma_start(out=t[:], in_=din[name].ap())
                return t

            idx_t = load("midx", (128, tot_slots // 16), mybir.dt.int16, mp)
            dstl_t = load("mdstl", (128, tot_chunks), f32, mp)
            dstr_t = load("mdstr", (128, tot_chunks), f32, mp)
            dinvd_t = load("mdinvd", (128, tot_chunks), f32, mp)
            self_t = load("mself", (128, _NSUB), f32)
            s0_t = load("ms0", (128, _NSUB), f32)
            s1_t = load("ms1", (128, _NSUB), f32)
            w0_t = load("w0", (64, 128), f32)
            wl_t = {nm: load(nm, (128, 128), f32) for nm in ["w1p", "w2p", "w3ap", "w3bp"]}
            b_t = {nm: load(nm, (128, 1), f32) for nm in bnames}
            tw1_t = load("tw1", (64, 64), f32)
            tw2_t = load("tw2", (64, 64), f32)
            tb1_t = load("tb1", (64, 1), f32)
            tb2_t = load("tb2", (64, 1), f32)
            e0_t = load("e0row", (1, 64), f32)
            e1_t = load("e1row", (1, 64), f32)
            rsin_t = load("rsin", (32, 1), f32)
            rcos_t = load("rcos", (32, 1), f32)
            ones1_t = load("ones1", (1, 128), f32)

            iota_i = cp.tile([128, 256], mybir.dt.int32, tag="iotai", name="iotai")
            nc.gpsimd.iota(iota_i[:], pattern=[[1, 256]], base=0, channel_multiplier=0)
            iota_b = cp.tile([128, 256], bf16, tag="iotab", name="iotab")
            nc.vector.tensor_copy(iota_b[:], iota_i[:])
            eye_t = cp.tile([128, 128], f32, tag="eye", name="eye")
            make_identity(nc, eye_t[:])

            # ---- DRAM working buffers ----
            slice_d = [dram.tile([_SLP, _F], bf16, tag=f"slice{l}", name=f"slice{l}") for l in range(4)]
            full_d = [dram.tile([_NP, _F], bf16, tag=f"full{l}", name=f"full{l}") for l in range(4)]
            h0T_d = dram.tile([128, _SLP], f32, tag="h0T", name="h0T")

            # ---- timestep embedding ----
            sc_t = sp.tile([64, 1], f32, tag="tsc", name="tsc")
            nc.scalar.activation(sc_t[:32, :], rsin_t[:], AT.Sin)
            nc.scalar.activation(sc_t[32:64, :], rcos_t[:], AT.Sin)
            h1ps = psC.tile([64, 1], f32, tag="temb", name="h1ps")
            nc.tensor.matmul(h1ps[:], lhsT=tw1_t[:], rhs=sc_t[:], start=True, stop=True)
            h1_t = sp.tile([64, 1], f32, tag="th1", name="th1")
            nc.scalar.activation(h1_t[:], h1ps[:], AT.Silu, bias=tb1_t[:, :1])
            t2ps = psC.tile([64, 1], f32, tag="temb", name="t2ps")
            nc.tensor.matmul(t2ps[:], lhsT=tw2_t[:], rhs=h1_t[:], start=True, stop=True)
            tembT = sp.tile([64, 1], f32, tag="tembT", name="tembT")
            nc.vector.tensor_scalar(out=tembT[:], in0=t2ps[:], scalar1=tb2_t[:, :1],
                                    scalar2=None, op0=OP.add)
            trow_ps = psC.tile([1, 64], f32, tag="temb", name="trow_ps")
            nc.tensor.transpose(trow_ps[:], in_=tembT[:], identity=eye_t[:64, :64])
            trow_t = sp.tile([1, 64], f32, tag="trowS", name="trowS")
            nc.vector.tensor_copy(trow_t[:], trow_ps[:])
            rows_ps = psC.tile([128, 192], f32, tag="temb", name="rows_ps")
            nc.tensor.matmul(rows_ps[:, 0:64], lhsT=ones1_t[:], rhs=trow_t[:],
                             start=True, stop=True, skip_group_check=True)
            nc.tensor.matmul(rows_ps[:, 64:128], lhsT=ones1_t[:], rhs=e0_t[:],
                             start=True, stop=True, skip_group_check=True)
            nc.tensor.matmul(rows_ps[:, 128:192], lhsT=ones1_t[:], rhs=e1_t[:],
                             start=True, stop=True, skip_group_check=True)
            addrows = cp.tile([128, 192], f32, tag="addrows", name="addrows")
            nc.vector.tensor_copy(addrows[:], rows_ps[:])

            # ---- x0 phase: Hs0 = dinv * ((noise + temb + lab) @ w0) ----
            for st in range(_NSUB):
                nz = sp.tile([128, 64], f32, tag="nz", name="nz")
                nc.sync.dma_start(out=nz[:], in_=din["noise"].ap()[st * 128:(st + 1) * 128, :])
                x0 = sp.tile([128, 64], f32, tag="x0", name="x0")
                nc.vector.tensor_add(x0[:], nz[:], addrows[:, 0:64])
                lab = sp.tile([128, 64], f32, tag="lab", name="lab")
                nc.vector.tensor_scalar(out=lab[:], in0=addrows[:, 64:128],
                                        scalar1=s0_t[:, st:st + 1], scalar2=None, op0=OP.mult)
                nc.vector.tensor_add(x0[:], x0[:], lab[:])
                nc.vector.tensor_scalar(out=lab[:], in0=addrows[:, 128:192],
                                        scalar1=s1_t[:, st:st + 1], scalar2=None, op0=OP.mult)
                nc.vector.tensor_add(x0[:], x0[:], lab[:])
                x0T_ps = psB.tile([64, 128], f32, tag="mm128", name="x0T_ps")
                nc.tensor.transpose(x0T_ps[:], in_=x0[:], identity=eye_t[:])
                x0T = sp.tile([64, 128], f32, tag="x0Ts", name="x0Ts")
                nc.vector.tensor_copy(x0T[:], x0T_ps[:])
                hps = psB.tile([128, 128], f32, tag="mm128", name="hps")
                nc.tensor.matmul(hps[:], lhsT=x0T[:], rhs=w0_t[:], start=True, stop=True)
                hs0 = sp.tile([128, _F], bf16, tag="hsout", name="hsout")
                nc.vector.tensor_scalar(out=hs0[:], in0=hps[:],
                                        scalar1=self_t[:, st:st + 1], scalar2=None, op0=OP.mult)
                nc.sync.dma_start(out=slice_d[0][st * 128:(st + 1) * 128, :], in_=hs0[:])

            # ---- layers ----
            for layer in range(4):
                nc.gpsimd.collective_compute(
                    "AllGather", mybir.AluOpType.bypass,
                    replica_groups=[list(range(_NCORES))],
                    ins=[slice_d[layer].opt()], outs=[full_d[layer].opt()],
                )
                full = full_d[layer]
                for w in range(_NWIN):
                    ws = min(_WIN, _SLP - w * _WIN)
                    ncw = int(nchunks_w[w])
                    cbase = int(np.sum(nchunks_w[:w]))
                    g = gp.tile([128, ncw, _F], bf16, tag="g", name="g")
                    # gather calls: per bucket, split into <=4-chunk calls
                    crel = 0
                    for b in range(_NBUCK):
                        nch = int(cpb[w, b])
                        o16 = int(call_off[w, b]) // 16
                        done = 0
                        while done < nch:
                            cc = min(_MAX_CHUNKS_PER_CALL, nch - done)
                            ni = cc * 128
                            nc.gpsimd.dma_gather(
                                out_ap=g[:, crel + done:crel + done + cc, :],
                                in_ap=full[b * _BUCK:(b + 1) * _BUCK, :],
                                idxs_ap=idx_t[:, o16 + done * 8: o16 + done * 8 + ni // 16],
                                num_idxs=ni, num_idxs_reg=ni, elem_size=_F,
                                queue_num=next_q(),
                            )
                            done += cc
                        crel += nch
                    agg = psA.tile([128, ws], f32, tag="agg", name="agg")
                    for c in range(ncw):
                        gc = cbase + c
                        oh = ohp.tile([128, ws], bf16, tag="oh", name="oh")
                        nc.vector.tensor_scalar(
                            out=oh[:, 0:256], in0=iota_b[:],
                            scalar1=dstl_t[:, gc:gc + 1], scalar2=dinvd_t[:, gc:gc + 1],
                            op0=OP.is_equal, op1=OP.mult)
                        if ws > 256:
                            nc.vector.tensor_scalar(
                                out=oh[:, 256:512], in0=iota_b[:],
                                scalar1=dstr_t[:, gc:gc + 1], scalar2=dinvd_t[:, gc:gc + 1],
                                op0=OP.is_equal, op1=OP.mult)
                        nc.tensor.matmul(agg[:], lhsT=g[:, c, :], rhs=oh[:],
                                         start=(c == 0), stop=False,
                                         skip_group_check=True)
                    for st in range(ws // 128):
                        gst = w * 4 + st
                        hself = sp.tile([128, _F], bf16, tag="hself", name="hself")
                        nc.sync.dma_start(
                            out=hself[:],
                            in_=slice_d[layer][(w * _WIN + st * 128):(w * _WIN + st * 128 + 128), :])
                        srhs = ohp.tile([128, 128], bf16, tag="srhs", name="srhs")
                        nc.vector.tensor_scalar(out=srhs[:], in0=eye_t[:],
                                                scalar1=self_t[:, gst:gst + 1],
                                                scalar2=None, op0=OP.mult)
                        nc.tensor.matmul(agg[:, st * 128:(st + 1) * 128],
                                         lhsT=hself[:], rhs=srhs[:],
                                         start=False, stop=(st == ws // 128 - 1),
                                         skip_group_check=True)
                    xT = xtp.tile([128, ws], f32, tag="xT", name="xT")
                    nc.scalar.activation(xT[:], agg[:], AT.Silu, bias=b_t[bnames[layer]][:, :1])
                    if layer == 0:
                        nc.sync.dma_start(out=h0T_d[:, w * _WIN:w * _WIN + ws], in_=xT[:])
                    if layer < 3:
                        for st in range(ws // 128):
                            gst = w * 4 + st
                            hps = psB.tile([128, 128], f32, tag="mm128", name="hps2")
                            if layer < 2:
                                nc.tensor.matmul(hps[:], lhsT=xT[:, st * 128:(st + 1) * 128],
                                                 rhs=wl_t[wnames[layer]][:], start=True, stop=True)
                            else:
                                nc.tensor.matmul(hps[:], lhsT=xT[:, st * 128:(st + 1) * 128],
                                                 rhs=wl_t["w3ap"][:], start=True, stop=False,
                                                 skip_group_check=True)
                                h0tile = sp.tile([128, 128], f32, tag="h0tile", name="h0tile")
                                nc.sync.dma_start(
                                    out=h0tile[:],
                                    in_=h0T_d[:, (w * _WIN + st * 128):(w * _WIN + st * 128 + 128)])
                                nc.tensor.matmul(hps[:], lhsT=h0tile[:], rhs=wl_t["w3bp"][:],
                                                 start=False, stop=True, skip_group_check=True)
                            hs = sp.tile([128, _F], bf16, tag="hsout", name="hsout")
                            nc.vector.tensor_scalar(out=hs[:], in0=hps[:],
                                                    scalar1=self_t[:, gst:gst + 1],
                                                    scalar2=None, op0=OP.mult)
                            nc.sync.dma_start(
                                out=slice_d[layer + 1][(w * _WIN + st * 128):(w * _WIN + st * 128 + 128), :],
                                in_=hs[:])
                    else:
                        for st in range(ws // 128):
                            ops = psB.tile([128, 128], f32, tag="mm128", name="ops")
                            nc.tensor.transpose(ops[:], in_=xT[:, st * 128:(st + 1) * 128],
                                                identity=eye_t[:])
                            oc = sp.tile([128, 64], f32, tag="outt", name="outt")
                            nc.vector.tensor_copy(oc[:], ops[:, 0:64])
                            nc.sync.dma_start(
                                out=out_d.ap()[(w * _WIN + st * 128):(w * _WIN + st * 128 + 128), :],
                                in_=oc[:])

    nc.compile()
    return nc


def _get_compiled(inputs):
    in_maps, cpb, call_off, nchunks_w, tot_chunks, tot_slots = _prep(inputs)
    key = cpb.tobytes()
    if key not in _compiled:
        _compiled[key] = _build(cpb, call_off, nchunks_w, tot_chunks, tot_slots)
    return _compiled[key], in_maps


def _run(inputs, trace=False):
    _install_profile_shim()
    from concourse import bass_utils
    nc, in_maps = _get_compiled(inputs)
    res = bass_utils.run_bass_kernel_spmd(
        nc, in_maps, core_ids=list(range(_NCORES)), trace=trace)
    out = np.concatenate([res.results[k]["out"][:_SL] for k in range(_NCORES)], axis=0)
    return out[:_N].astype(np.float32), res.exec_time_ns


def kernel(**inputs):
    out, _ = _run(inputs, trace=False)
    return out
